# revision 33
# baseline (speedup 1.0000x reference)
"""Trainium2 Bass kernel for nn_LSM_30176440221725 (latent-space-model loss).

LL = sum_e [beta_ie + gamma_je - ||zi_ie - zj_je + eps||]          (link term)
     - sum_{i in Si, j in Sj} exp(beta_i + gamma_j - ||zi_i - zj_j + eps||)

Sharding (8 cores): sample_i rows of the [Si,Sj] pairwise block are sharded
across cores (each core holds the full sample_j side); the 500k-edge link
term is sharded by edge. Per-core partials ([128,5]) are combined on host.

v2 design, 22937ns vs the 25966ns v1 (ACT was 77%-busy there; now ACT/DVE
are co-critical at ~63-69%):
 - pair d2 via K=12 bf16 matmuls (hi/lo |z|^2 splits) as v1, but split by
   j-column class: h0 cols [0:1536] go to [128,1536] psum tiles that ACT
   Sqrts (scale 2^-60, so u' = dist*2^-30; the Sqrt table breaks below
   scale ~2^-100); h1 cols [1536:3000] go to [128,512] psum blocks that
   DVE "quake"-sqrts: ONE tensor_scalar shift-right-1 on the high halfword
   of the f32 psum ([:, 1::2, None] AP - a plain stride-2 slice hard-faults
   the device, and the verifier rejects shift+add fusion and u32->u16
   narrowing). value(bits>>1) = dist/K with K = 2^63.475 * sawtooth(+-4%);
   the host prescales the h1 za columns by G = 2^6.953*2^60 so the quake
   output lands exactly at dist*2^-30 with zero exp-weighted mean error -
   the magic-add constant is just a multiply in float domain, so it folds
   into G and the unified Exp scale with zero extra device ops.
 - subs s = u' + (-gamma*2^-30): one TT-add form for ALL columns, split
   DVE (bf16 2x: c0a,c0b,c1b,c2b-tail) / Pool (gpsimd: c1a,c2a,c2b-head)
   to meet each exp chunk's deadline.
 - 3 exps, one per i-chunk, unified scale -2^30, per-chunk beta bias,
   accum_out row sums into R (the free reduction).
 - link term in dot form: d2 = qi + qj - 2*zi.zj with node-level qi/qj and
   -2*zj prescaled on host (O(N*D) prep, gathered per edge); DVE does
   mult + tree + quake-shift (2x) + reduces; Pool does qi+qj and
   beta+gamma; host applies K_LINK to the raw dist sum.
 - matmul emission c0h0, c1h0, h1(0), c2h0, h1(1), h1(2) keeps the ACT
   sqrt phase gapless (the scheduler is readiness-driven: if an exp is
   ready before the last sqrt, it runs first and thrashes the activation
   table, +2.5us) while feeding DVE quakes early. First sqrt is gated on
   psum bank 0 only (psum deps are bank-granular).
Host does gather/shard/pad/cast plus node-level O(N*D)/O(S*D) scalar prep;
all O(Si*Sj) and O(E*D) math is on device.
"""
import sys

sys.path.insert(0, "/opt/trn_rl_repo")

import numpy as np

EPS = 1e-6
N_I = N_J = 100000
S_I = S_J = 3000
N_LINKS = 500000
NCORES = 8

SPC = S_I // NCORES              # 375 sample_i rows per core
CI = 3                           # i chunks of 128 (375 -> 384)
NJ = 3072                        # U stride per chunk
SJ = S_J                         # 3000 real j columns
HALF = 1536                      # h0 psum tile width (3 banks)
QB = 512                         # h1 quake psum block width (1 bank)
EPC = N_LINKS // NCORES          # 62500 edges per core
CL = (EPC + 127) // 128          # 489 columns of 128 edges
NC = 20                          # link comps: zi(8) zj2(8) qi qj beta gamma
G_QK = 1.238862648e2 * 2.0 ** 60  # 2^66.9529: host prescale on quake-class
                                 # d2 (the ACT Sqrt table breaks below scale
                                 # ~2^-100, so target exponent E=30 not 60)
SC_SQ = 2.0 ** -60               # ACT sqrt scale: u' = sqrt(d2)*2^-30
B_SQ = 0.03 * 2.0 ** -60         # NaN-guard bias in the scaled domain
KE = 2.0 ** 30                   # unified exp scale (-KE) for both classes
K_LINK = 1.2778381718895426e19   # link quake scale (host-applied)
A = 1536                         # ACT-sqrt / exp-class column split

_CACHE = {}


def _build_program():
    import concourse.bass as bass
    import concourse.bacc as bacc
    import concourse.tile as tile
    from concourse import mybir

    f32 = mybir.dt.float32
    bf16 = mybir.dt.bfloat16
    u16 = mybir.dt.uint16
    AF = mybir.ActivationFunctionType
    ALU = mybir.AluOpType

    nc = bacc.Bacc("TRN2", target_bir_lowering=False, debug=False)

    # za cols 0:384 = lhsT (rows 0-7 zi, 8-9 ones, 10-11 qi_hi/qi_lo),
    # cols 384:3456 = rhs (rows 0-7 -2*zj, 8-9 qj_hi/qj_lo, 10-11 ones)
    za = nc.dram_tensor("za", [12, 384 + NJ], bf16, kind="ExternalInput")
    grow = nc.dram_tensor("grow", [128, SJ], bf16, kind="ExternalInput")
    bq = nc.dram_tensor("bq", [128, CI], f32, kind="ExternalInput")
    eall = nc.dram_tensor("eall", [128, CL, 16], bf16, kind="ExternalInput")
    eal2 = nc.dram_tensor("eal2", [128, CL, 6], bf16, kind="ExternalInput")
    rout = nc.dram_tensor("rout", [128, 5], f32, kind="ExternalOutput")

    with tile.TileContext(nc) as tc:
        with tc.tile_pool(name="main", bufs=1) as mp, \
             tc.tile_pool(name="psD", bufs=2, space="PSUM") as psD, \
             tc.tile_pool(name="psQ", bufs=2, space="PSUM") as psQ:

            # ---- operand loads on the serial DMA device ----
            za_t = mp.tile([12, 384 + NJ], bf16)
            nc.sync.dma_start(out=za_t[:], in_=za[:])
            bc = mp.tile([128, CI], f32)
            nc.sync.dma_start(out=bc[:], in_=bq[:])
            gbc = mp.tile([128, SJ], bf16)
            nc.sync.dma_start(out=gbc[:], in_=grow[:])
            et = mp.tile([128, CL, 16], bf16)
            nc.sync.dma_start(out=et[:], in_=eall[:])
            # qi/qj/beta/gamma land last (padded to 6 comps so the transfer
            # ends after the c2b sub is ready: the Pool then picks the
            # deadline-critical sub over qs/csum at its ~13.3us idle slot)
            et2 = mp.tile([128, CL, 6], bf16)
            nc.sync.dma_start(out=et2[:], in_=eal2[:])

            bias_sq = mp.tile([128, 1], f32)
            nc.vector.memset(bias_sq[:], B_SQ)
            R = mp.tile([128, 5], f32)

            U = mp.tile([128, CI * NJ], bf16)
            S = mp.tile([128, CI * SJ], bf16)
            U16 = U[:].bitcast(u16)

            def quake(dst0, ps_ap, w):
                ph = ps_ap.bitcast(u16)
                nc.vector.tensor_scalar(
                    out=U16[:, dst0:dst0 + w],
                    in0=ph[:, 1::2, None], scalar1=1, scalar2=None,
                    op0=ALU.logical_shift_right)

            def h0_tile(c):
                ps = psD.tile([128, HALF], f32, tag="d2")
                if c == 0:
                    blocks = [(0, 256), (256, 512), (512, 1024), (1024, 1536)]
                else:
                    blocks = [(0, 512), (512, 1024), (1024, 1536)]
                for b0, b1 in blocks:
                    nc.tensor.matmul(
                        out=ps[:, b0:b1],
                        lhsT=za_t[:, c * 128:(c + 1) * 128],
                        rhs=za_t[:, 384 + b0:384 + b1],
                        start=True, stop=True)
                u0 = c * NJ
                if c == 0:
                    with tc.high_priority():
                        nc.scalar.activation(out=U[:, u0:u0 + 256],
                                             in_=ps[:, 0:256], func=AF.Sqrt,
                                             bias=bias_sq[:, 0:1], scale=SC_SQ)
                    nc.scalar.activation(out=U[:, u0 + 256:u0 + A],
                                         in_=ps[:, 256:A], func=AF.Sqrt,
                                         bias=bias_sq[:, 0:1], scale=SC_SQ)
                else:
                    nc.scalar.activation(out=U[:, u0:u0 + A],
                                         in_=ps[:, 0:A], func=AF.Sqrt,
                                         bias=bias_sq[:, 0:1], scale=SC_SQ)
                if A < HALF:
                    quake(c * NJ + A, ps[:, A:HALF], HALF - A)

            def h1_blocks(c):
                for k in range(3):
                    b0 = HALF + k * QB
                    b1 = min(b0 + QB, SJ)
                    ps = psQ.tile([128, QB], f32, tag="q")
                    nc.tensor.matmul(
                        out=ps[:, 0:b1 - b0],
                        lhsT=za_t[:, c * 128:(c + 1) * 128],
                        rhs=za_t[:, 384 + b0:384 + b1],
                        start=True, stop=True)
                    quake(c * NJ + b0, ps[:, 0:b1 - b0], b1 - b0)

            def sub(c, j0, j1, eng):
                eng.tensor_tensor(
                    out=S[:, c * SJ + j0:c * SJ + j1],
                    in0=U[:, c * NJ + j0:c * NJ + j1],
                    in1=gbc[:, j0:j1], op=ALU.add)

            # front-load h0 tiles (gapless ACT sqrt phase), interleave h1;
            # subs (s = u + (-gamma/KE)) emitted where their inputs land:
            # DVE c0a/c0b/c1b/c2b-tail, Pool c1a/c2a/c2b-head
            h0_tile(0)
            h0_tile(1)
            h1_blocks(0)
            sub(0, 0, A, nc.vector)
            sub(0, A, SJ, nc.vector)
            sub(1, 0, A, nc.gpsimd)
            h0_tile(2)
            h1_blocks(1)
            sub(1, A, SJ, nc.vector)
            sub(2, 0, A, nc.gpsimd)
            h1_blocks(2)
            sub(2, A, 2304, nc.gpsimd)
            sub(2, 2304, SJ, nc.vector)

            # ---- link: d2 = (qi+qj) - 2*zi.zj, quake sqrt, reduces ----
            M = mp.tile([128, CL, 8], bf16)
            S1 = mp.tile([128, CL, 4], bf16)
            S2 = mp.tile([128, CL, 2], bf16)
            S3 = mp.tile([128, CL], bf16)
            QS = mp.tile([128, CL], bf16)
            D2L = mp.tile([128, CL], bf16)
            DL = mp.tile([128, CL], u16)
            CS = mp.tile([128, CL], bf16)

            nc.vector.tensor_tensor(out=M[:], in0=et[:, :, 0:8],
                                    in1=et[:, :, 8:16], op=ALU.mult)
            nc.vector.tensor_tensor(out=S1[:], in0=M[:, :, 0:4],
                                    in1=M[:, :, 4:8], op=ALU.add)
            nc.vector.tensor_tensor(out=S2[:], in0=S1[:, :, 0:2],
                                    in1=S1[:, :, 2:4], op=ALU.add)
            nc.vector.tensor_tensor(out=S3[:], in0=S2[:, :, 0],
                                    in1=S2[:, :, 1], op=ALU.add)
            nc.gpsimd.tensor_tensor(out=QS[:], in0=et2[:, :, 0],
                                    in1=et2[:, :, 1], op=ALU.add)
            nc.vector.tensor_tensor(out=D2L[:], in0=S3[:],
                                    in1=QS[:], op=ALU.add)
            nc.vector.tensor_scalar(out=DL[:], in0=D2L[:].bitcast(u16),
                                    scalar1=1, scalar2=None,
                                    op0=ALU.logical_shift_right)
            nc.vector.tensor_reduce(out=R[:, 3:4], in_=DL[:].bitcast(bf16),
                                    axis=mybir.AxisListType.X, op=ALU.add)
            # beta+gamma on Pool, reduced on DVE
            nc.gpsimd.tensor_tensor(out=CS[:], in0=et2[:, :, 2],
                                    in1=et2[:, :, 3], op=ALU.add)
            nc.vector.tensor_reduce(out=R[:, 4:5], in_=CS[:],
                                    axis=mybir.AxisListType.X, op=ALU.add)


            # ---- pair: exps (unified -KE scale) with accum row sums ----
            Tdump = mp.tile([128, SJ], bf16)
            for c in range(CI):
                nc.scalar.activation(
                    out=Tdump[:], in_=S[:, c * SJ:(c + 1) * SJ], func=AF.Exp,
                    bias=bc[:, c:c + 1], scale=-KE, accum_out=R[:, c:c + 1])

            nc.sync.dma_start(out=rout[:], in_=R[:])
    nc.compile()
    return nc


def _host_prep(latent_zi, latent_zj, beta, gamma,
               sample_i_idx, sample_j_idx, sparse_i_sample, sparse_j_sample):
    """Gather/shard/pad/cast + node-level O(N*D)/O(S*D) scalar prep."""
    latent_zi = np.asarray(latent_zi, np.float32)
    latent_zj = np.asarray(latent_zj, np.float32)
    beta = np.asarray(beta, np.float32)
    gamma = np.asarray(gamma, np.float32)
    si = np.asarray(sample_i_idx).astype(np.int64)
    sj = np.asarray(sample_j_idx).astype(np.int64)
    li = np.asarray(sparse_i_sample).astype(np.int64)
    lj = np.asarray(sparse_j_sample).astype(np.int64)

    from concourse import mybir
    bf = mybir.dt.np(mybir.dt.bfloat16)

    zi_s = latent_zi[si]                     # [3000, 8]
    b_s = beta[si]
    zj_s = latent_zj[sj]                     # [3000, 8]
    g_s = gamma[sj]
    qi = (zi_s * zi_s).sum(1) + 2 * EPS * zi_s.sum(1)
    qj = (zj_s * zj_s).sum(1) - 2 * EPS * zj_s.sum(1) + 8 * EPS * EPS
    gneg = (-g_s / KE).astype(np.float32)
    grow = np.broadcast_to(gneg, (128, S_J)).astype(bf)

    # node-level link prep (gathered per edge below)
    zj2_nodes = (-2.0 * latent_zj).astype(bf)
    qi_nodes = ((latent_zi * latent_zi).sum(1)
                + 2 * EPS * latent_zi.sum(1)).astype(np.float32)
    qj_nodes = ((latent_zj * latent_zj).sum(1)
                - 2 * EPS * latent_zj.sum(1) + 8 * EPS * EPS).astype(np.float32)

    in_maps = []
    for c in range(NCORES):
        s0 = c * SPC
        qic = qi[s0:s0 + SPC]
        qi_hi = qic.astype(bf).astype(np.float32)
        qi_lo = qic - qi_hi
        za = np.zeros((12, 384 + NJ), np.float32)
        za[0:8, :SPC] = zi_s[s0:s0 + SPC].T
        za[8, :384] = 1.0
        za[9, :384] = 1.0
        za[10, :SPC] = qi_hi
        za[11, :SPC] = qi_lo
        # rhs columns: quake-class cols j>=A carry the G_QK prescale so the
        # shifted psum bits read as dist*2^-60 (matches the unified exp scale)
        colg = np.where(np.arange(SJ) < A, 1.0, G_QK).astype(np.float32)
        za[0:8, 384:384 + SJ] = (-2.0 * zj_s).T * colg
        qjg = qj * colg
        qjg_hi = qjg.astype(bf).astype(np.float32)
        za[8, 384:384 + SJ] = qjg_hi
        za[9, 384:384 + SJ] = qjg - qjg_hi
        za[10, 384:384 + SJ] = colg
        za[11, 384:384 + SJ] = colg

        bflat = np.full(CI * 128, -1e30, np.float32)
        bflat[:SPC] = b_s[s0:s0 + SPC]
        bcol = bflat.reshape(CI, 128).T.copy()

        e0 = c * EPC
        es = np.zeros((128 * CL, NC), np.float32)
        idx_i = li[e0:e0 + EPC]
        idx_j = lj[e0:e0 + EPC]
        es[:EPC, 0:8] = latent_zi[idx_i]
        es[:EPC, 8:16] = zj2_nodes[idx_j]
        es[:EPC, 16] = qi_nodes[idx_i]
        es[:EPC, 17] = qj_nodes[idx_j]
        es[:EPC, 18] = beta[idx_i]
        es[:EPC, 19] = gamma[idx_j]
        ec = es.reshape(CL, 128, NC).transpose(1, 0, 2).astype(bf)

        e2 = np.zeros((128, CL, 6), ec.dtype)
        e2[:, :, 0:4] = ec[:, :, 16:20]
        in_maps.append({"za": za.astype(bf), "grow": grow, "bq": bcol,
                        "eall": np.ascontiguousarray(ec[:, :, 0:16]),
                        "eal2": e2})
    return in_maps


def kernel(**inputs):
    from concourse import bass_utils

    if "nc" not in _CACHE:
        _CACHE["nc"] = _build_program()
    nc = _CACHE["nc"]
    in_maps = _host_prep(**inputs)
    res = bass_utils.run_bass_kernel_spmd(nc, in_maps, core_ids=list(range(NCORES)))
    total = np.float64(0.0)
    for c in range(NCORES):
        R = np.asarray(res.results[c]["rout"], np.float64)
        link = R[:, 4].sum() - K_LINK * R[:, 3].sum()
        total += link - R[:, 0:3].sum()
    return np.asarray(total, dtype=np.float32)
